# revision 33
# baseline (speedup 1.0000x reference)
"""DC feasibility layer (primal-dual projection) as a Trainium2 Bass kernel.

Problem: B=65536 samples x n_gen=256. 12 unrolled primal-dual iterations:
    p <- clip(p - 0.5*((p - p_pred) + lam), pmin, pmax)
    lam <- lam + 0.8*(sum_gens(p) - load)
then a final slack redistribution among strictly-interior generators.

The dual never converges -- it orbits a limit cycle and the trajectory is
chaotic, so the kernel reproduces the reference's f32 rounding BIT-EXACTLY
(verified vs a numpy model) everywhere except the row-sum accumulation
order. Trick: track W := 2*p (scaling by 2 commutes with every f32
rounding), which lets each iteration collapse into a few fused ops while
keeping the reference's exact rounding sequence:
    r1n = fl(a - 0.5*W)                 == -fl(p - p_pred)      [DVE, chunked]
    t   = fl(r1n + (-lam))              [ScalarE Identity+bias: bit-exact]
    Wr  = fl(t + W)                     == 2*fl(p - 0.5*grad)   [DVE, chunked]
    W   = min(max(Wr, 0), 2*pmax)  (+ fused row-sum accum)      [DVE/group]
The dual row-update runs per chunk of 16 groups, so chunks of iteration
k+1 pipeline behind chunks of iteration k with no global barrier. The
row-sum accumulation stays on the Vector engine: its hardware order was
verified to match jnp.sum bitwise on this data.

Sharding: pure data parallel over the batch across 8 NeuronCores
(8192 samples/core). On-chip layout: samples on partitions
(sample = g*128 + partition for group g < 64), gens on the free dim, so
lam is a per-partition scalar per group and the whole pipeline is
scalar_tensor_tensor / tensor_scalar ops on the Vector engine.

p_min is assumed identically zero (guaranteed by the problem's
setup_inputs); the kernel asserts this. LAM_CLIP at 1e6 is a provable
no-op (|lam| <= 12*0.8*sum(pmax) < 3e4) and is skipped.
"""
import numpy as np

try:
    import concourse.bacc as bacc
except ImportError:
    import sys
    sys.path.insert(0, "/opt/trn_rl_repo")
    import concourse.bacc as bacc
import concourse.bass as bass
import concourse.mybir as mybir
import concourse.tile as tile
from concourse.bass_utils import run_bass_kernel_spmd

F32 = mybir.dt.float32
OP = mybir.AluOpType

N_ITER = 12
SLACK_EPS = 1e-8

B_FULL = 65536
NG = 256
N_CORES = 8


def build_nc(G, num_devices, chunk=16, n_iter=N_ITER):
    """Build the per-core kernel for G groups of 128 samples."""
    FD = G * NG
    nch = max(G // chunk, 1)
    chunk = G // nch
    CFD = chunk * NG

    nc = bacc.Bacc("TRN2", target_bir_lowering=False, debug=False,
                   num_devices=num_devices)
    a_d = nc.dram_tensor("a", [128, FD], F32, kind="ExternalInput")
    ld_d = nc.dram_tensor("loadc", [128, G], F32, kind="ExternalInput")
    hi2_d = nc.dram_tensor("hib2", [128, NG], F32, kind="ExternalInput")
    thi2_d = nc.dram_tensor("thib2", [128, NG], F32, kind="ExternalInput")
    out_d = nc.dram_tensor("out", [128, FD], F32, kind="ExternalOutput")

    # 2*fl(pmin + eps): mask threshold in W-space (pmin == 0)
    TLO2 = float(np.float32(2.0) * np.float32(SLACK_EPS))

    with tile.TileContext(nc) as tc:
        with (
            tc.tile_pool(name="big", bufs=1) as big,
            tc.tile_pool(name="tch", bufs=(4 if chunk <= 16 else 2)) as tch,
            tc.tile_pool(name="grp", bufs=8) as grp,
            tc.tile_pool(name="rows", bufs=1) as rows,
        ):
            w = big.tile([128, FD], F32, tag="w")
            a = big.tile([128, FD], F32, tag="a")
            hib2 = rows.tile([128, NG], F32, tag="hib2")
            thib2 = rows.tile([128, NG], F32, tag="thib2")
            ldc = rows.tile([128, G], F32, tag="ldc")
            M2 = rows.tile([128, G], F32, tag="M2")
            scol = rows.tile([128, G], F32, tag="scol")
            sh = rows.tile([128, G], F32, tag="sh")
            rr = rows.tile([128, G], F32, tag="rr")
            ssum = rows.tile([128, G], F32, tag="ssum")
            dcol = rows.tile([128, G], F32, tag="dcol")
            dneg2 = rows.tile([128, G], F32, tag="dneg2")

            # Small constants first so iteration 1 isn't queued behind the
            # 8 MB input DMA.
            nc.sync.dma_start(ldc[:], ld_d[:])
            nc.sync.dma_start(hib2[:], hi2_d[:])
            nc.sync.dma_start(thib2[:], thi2_d[:])
            for c in range(nch):
                nc.sync.dma_start(a[:, c * CFD:(c + 1) * CFD],
                                  a_d[:, c * CFD:(c + 1) * CFD])

            def c3d(t, c):  # chunk c of a [128, FD] tile as (128, chunk, NG)
                return t[:, c * CFD:(c + 1) * CFD].rearrange(
                    "p (c f) -> p c f", c=chunk)

            hib2_b = hib2[:, :].rearrange("p (o f) -> p o f", o=1).to_broadcast(
                (128, chunk, NG))

            def gsl(g):
                return slice(g * NG, (g + 1) * NG)

            def lam_rows(first, c):
                # Per-chunk dual update on columns [c*chunk, (c+1)*chunk):
                # s = 0.5*scol ; r = fl(s - load) ; M2 (= -lam) -= fl(0.8*r).
                # Doing this per chunk (not per iteration) removes the global
                # iteration barrier -- chunk c of iteration k+1 only waits on
                # chunk c of iteration k, so chunks pipeline across iterations.
                cs = slice(c * chunk, (c + 1) * chunk)
                nc.vector.scalar_tensor_tensor(
                    out=rr[:, cs], in0=scol[:, cs], scalar=0.5,
                    in1=ldc[:, cs], op0=OP.mult, op1=OP.subtract)
                if first:
                    nc.vector.tensor_scalar(M2[:, cs], rr[:, cs], -0.8, None,
                                            OP.mult)
                else:
                    nc.vector.scalar_tensor_tensor(
                        out=M2[:, cs], in0=rr[:, cs], scalar=-0.8,
                        in1=M2[:, cs], op0=OP.mult, op1=OP.add)

            # ---- iteration 1: W = clip(2*a, 0, 2*pmax) ----
            for c in range(nch):
                nc.vector.scalar_tensor_tensor(
                    out=c3d(w, c), in0=c3d(a, c), scalar=2.0, in1=hib2_b,
                    op0=OP.mult, op1=OP.min)
                for gi in range(chunk):
                    g = c * chunk + gi
                    nc.vector.tensor_scalar(
                        out=w[:, gsl(g)], in0=w[:, gsl(g)], scalar1=0.0,
                        scalar2=0.0, op0=OP.max, op1=OP.add,
                        accum_out=scol[:, g:g + 1])
                lam_rows(True, c)

            # ---- iterations 2..12 ----
            for _ in range(n_iter - 1):
                for c in range(nch):
                    t = tch.tile([128, CFD], F32, tag="t")
                    # r1n = fl(a - 0.5*W)
                    nc.vector.scalar_tensor_tensor(
                        out=t[:, :].rearrange("p (c f) -> p c f", c=chunk),
                        in0=c3d(w, c), scalar=-0.5, in1=c3d(a, c),
                        op0=OP.mult, op1=OP.add)
                    for gi in range(chunk):
                        g = c * chunk + gi
                        ts = slice(gi * NG, (gi + 1) * NG)
                        # t = fl(r1n + M2) on ScalarE (bit-exact Identity)
                        nc.scalar.activation(
                            t[:, ts], t[:, ts],
                            mybir.ActivationFunctionType.Identity,
                            bias=M2[:, g:g + 1], scale=1.0)
                    # Wr = fl(t + W)   (in-place on W)
                    nc.vector.tensor_tensor(
                        c3d(w, c),
                        t[:, :].rearrange("p (c f) -> p c f", c=chunk),
                        c3d(w, c), OP.add)
                    for gi in range(chunk):
                        g = c * chunk + gi
                        # W = min(max(Wr, 0), 2*pmax), accum row-sum of W
                        nc.vector.scalar_tensor_tensor(
                            out=w[:, gsl(g)], in0=w[:, gsl(g)], scalar=0.0,
                            in1=hib2[:, :], op0=OP.max, op1=OP.min,
                            accum_out=scol[:, g:g + 1])
                    lam_rows(False, c)

            # ---- final slack redistribution ----
            # mask = (W > 2*(pmin+eps)) & (W < 2*(pmax-eps)); ssum = sum(mask)
            thib2_b = thib2[:, :].rearrange("p (o f) -> p o f",
                                            o=1).to_broadcast((128, chunk, NG))
            for c in range(nch):
                t1 = tch.tile([128, CFD], F32, tag="t")
                nc.vector.tensor_tensor(
                    t1[:, :].rearrange("p (c f) -> p c f", c=chunk),
                    c3d(w, c), thib2_b, OP.is_lt)
                for gi in range(chunk):
                    g = c * chunk + gi
                    ts = slice(gi * NG, (gi + 1) * NG)
                    nc.vector.scalar_tensor_tensor(
                        out=a[:, gsl(g)], in0=w[:, gsl(g)], scalar=TLO2,
                        in1=t1[:, ts], op0=OP.is_gt, op1=OP.mult,
                        accum_out=ssum[:, g:g + 1])
            # d = fl(res / max(ssum,1)) with res = fl(s12 - load) (== rr)
            nc.vector.tensor_scalar(ssum[:], ssum[:], 1.0, None, OP.max)
            nc.vector.reciprocal(ssum[:], ssum[:])
            nc.vector.tensor_tensor(dcol[:], rr[:], ssum[:], OP.mult)
            nc.vector.tensor_scalar(dneg2[:], dcol[:], -1.0, None, OP.mult)
            # p_out = fl(0.5*W + (-d)*mask). (-d)*mask is exact (mask is
            # 0/1, on ScalarE via per-partition scale); 0.5*W is exact, so one
            # fused STT reproduces the reference's fl(p - (res/ssum)*mask).
            for g in range(G):
                nc.scalar.activation(
                    a[:, gsl(g)], a[:, gsl(g)],
                    mybir.ActivationFunctionType.Copy,
                    scale=dneg2[:, g:g + 1])
            for c in range(nch):
                nc.vector.scalar_tensor_tensor(
                    out=c3d(w, c), in0=c3d(w, c), scalar=0.5, in1=c3d(a, c),
                    op0=OP.mult, op1=OP.add)
                nc.sync.dma_start(out_d[:, c * CFD:(c + 1) * CFD],
                                  w[:, c * CFD:(c + 1) * CFD])
    nc.compile()
    return nc


_NC_CACHE = {}


def _get_nc(G, num_devices):
    key = (G, num_devices)
    if key not in _NC_CACHE:
        _NC_CACHE[key] = build_nc(G, num_devices)
    return _NC_CACHE[key]


def _prep_inputs(p_pred, total_load, p_max, n_cores, G):
    f = np.float32
    a = np.ascontiguousarray(
        p_pred.astype(f, copy=False).reshape(n_cores, G, 128, NG)
        .transpose(0, 2, 1, 3)).reshape(n_cores, 128, G * NG)
    ld = np.ascontiguousarray(
        total_load.astype(f, copy=False).reshape(n_cores, G, 128)
        .transpose(0, 2, 1))
    pm = p_max.astype(f, copy=False)
    hib2 = np.ascontiguousarray(
        np.broadcast_to((f(2) * pm)[None, :], (128, NG)))
    thib2 = np.ascontiguousarray(
        np.broadcast_to((f(2) * (pm - f(SLACK_EPS)))[None, :], (128, NG)))
    return [dict(a=a[c], loadc=ld[c], hib2=hib2, thib2=thib2)
            for c in range(n_cores)]


def _unpack(results, n_cores, G):
    outs = [r["out"].reshape(128, G, NG).transpose(1, 0, 2).reshape(G * 128, NG)
            for r in results]
    return np.ascontiguousarray(np.concatenate(outs, axis=0))


def run(p_pred, total_load, p_min, p_max, **spmd_kwargs):
    assert p_pred.shape == (B_FULL, NG)
    assert not np.any(p_min), "kernel is specialized for p_min == 0"
    G = B_FULL // (N_CORES * 128)
    nc = _get_nc(G, N_CORES)
    in_maps = _prep_inputs(p_pred, total_load, p_max, N_CORES, G)
    res = run_bass_kernel_spmd(nc, in_maps, list(range(N_CORES)), **spmd_kwargs)
    return _unpack(res.results, N_CORES, G), res


def kernel(p_pred, total_load, p_min, p_max):
    return run(p_pred, total_load, p_min, p_max)[0]


# revision 34
# speedup vs baseline: 1.0002x; 1.0002x over previous
"""DC feasibility layer (primal-dual projection) as a Trainium2 Bass kernel.

Problem: B=65536 samples x n_gen=256. 12 unrolled primal-dual iterations:
    p <- clip(p - 0.5*((p - p_pred) + lam), pmin, pmax)
    lam <- lam + 0.8*(sum_gens(p) - load)
then a final slack redistribution among strictly-interior generators.

The dual never converges -- it orbits a limit cycle and the trajectory is
chaotic, so the kernel reproduces the reference's f32 rounding BIT-EXACTLY
(verified vs a numpy model) everywhere except the row-sum accumulation
order. Trick: track W := 2*p (scaling by 2 commutes with every f32
rounding), which lets each iteration collapse into a few fused ops while
keeping the reference's exact rounding sequence:
    r1n = fl(a - 0.5*W)                 == -fl(p - p_pred)      [DVE, chunked]
    t   = fl(r1n + (-lam))              [ScalarE Identity+bias: bit-exact]
    Wr  = fl(t + W)                     == 2*fl(p - 0.5*grad)   [DVE, chunked]
    W   = min(max(Wr, 0), 2*pmax)  (+ fused row-sum accum)      [DVE/group]
The dual row-update runs per chunk of 16 groups, so chunks of iteration
k+1 pipeline behind chunks of iteration k with no global barrier. The
row-sum accumulation stays on the Vector engine: its hardware order was
verified to match jnp.sum bitwise on this data.

Sharding: pure data parallel over the batch across 8 NeuronCores
(8192 samples/core). On-chip layout: samples on partitions
(sample = g*128 + partition for group g < 64), gens on the free dim, so
lam is a per-partition scalar per group and the whole pipeline is
scalar_tensor_tensor / tensor_scalar ops on the Vector engine.

p_min is assumed identically zero (guaranteed by the problem's
setup_inputs); the kernel asserts this. LAM_CLIP at 1e6 is a provable
no-op (|lam| <= 12*0.8*sum(pmax) < 3e4) and is skipped.
"""
import numpy as np

try:
    import concourse.bacc as bacc
except ImportError:
    import sys
    sys.path.insert(0, "/opt/trn_rl_repo")
    import concourse.bacc as bacc
import concourse.bass as bass
import concourse.mybir as mybir
import concourse.tile as tile
from concourse.bass_utils import run_bass_kernel_spmd

F32 = mybir.dt.float32
OP = mybir.AluOpType

N_ITER = 12
SLACK_EPS = 1e-8

B_FULL = 65536
NG = 256
N_CORES = 8


def build_nc(G, num_devices, chunk=16, n_iter=N_ITER):
    """Build the per-core kernel for G groups of 128 samples."""
    FD = G * NG
    nch = max(G // chunk, 1)
    chunk = G // nch
    CFD = chunk * NG

    nc = bacc.Bacc("TRN2", target_bir_lowering=False, debug=False,
                   num_devices=num_devices)
    a_d = nc.dram_tensor("a", [128, FD], F32, kind="ExternalInput")
    ld_d = nc.dram_tensor("loadc", [128, G], F32, kind="ExternalInput")
    hi2_d = nc.dram_tensor("hib2", [128, NG], F32, kind="ExternalInput")
    thi2_d = nc.dram_tensor("thib2", [128, NG], F32, kind="ExternalInput")
    out_d = nc.dram_tensor("out", [128, FD], F32, kind="ExternalOutput")

    # 2*fl(pmin + eps): mask threshold in W-space (pmin == 0)
    TLO2 = float(np.float32(2.0) * np.float32(SLACK_EPS))

    with tile.TileContext(nc) as tc:
        with (
            tc.tile_pool(name="big", bufs=1) as big,
            tc.tile_pool(name="tch", bufs=(4 if chunk <= 16 else 2)) as tch,
            tc.tile_pool(name="grp", bufs=8) as grp,
            tc.tile_pool(name="rows", bufs=1) as rows,
        ):
            w = big.tile([128, FD], F32, tag="w")
            a = big.tile([128, FD], F32, tag="a")
            hib2 = rows.tile([128, NG], F32, tag="hib2")
            thib2 = rows.tile([128, NG], F32, tag="thib2")
            ldc = rows.tile([128, G], F32, tag="ldc")
            M2 = rows.tile([128, G], F32, tag="M2")
            scol = rows.tile([128, G], F32, tag="scol")
            sh = rows.tile([128, G], F32, tag="sh")
            rr = rows.tile([128, G], F32, tag="rr")
            ssum = rows.tile([128, G], F32, tag="ssum")
            dcol = rows.tile([128, G], F32, tag="dcol")
            dneg2 = rows.tile([128, G], F32, tag="dneg2")

            # Small constants first so iteration 1 isn't queued behind the
            # 8 MB input DMA.
            nc.sync.dma_start(ldc[:], ld_d[:])
            nc.sync.dma_start(hib2[:], hi2_d[:])
            nc.sync.dma_start(thib2[:], thi2_d[:])
            for c in range(nch):
                nc.sync.dma_start(a[:, c * CFD:(c + 1) * CFD],
                                  a_d[:, c * CFD:(c + 1) * CFD])

            def c3d(t, c):  # chunk c of a [128, FD] tile as (128, chunk, NG)
                return t[:, c * CFD:(c + 1) * CFD].rearrange(
                    "p (c f) -> p c f", c=chunk)

            hib2_b = hib2[:, :].rearrange("p (o f) -> p o f", o=1).to_broadcast(
                (128, chunk, NG))

            def gsl(g):
                return slice(g * NG, (g + 1) * NG)

            def lam_rows(first, c):
                # Per-chunk dual update on columns [c*chunk, (c+1)*chunk):
                # s = 0.5*scol ; r = fl(s - load) ; M2 (= -lam) -= fl(0.8*r).
                # Doing this per chunk (not per iteration) removes the global
                # iteration barrier -- chunk c of iteration k+1 only waits on
                # chunk c of iteration k, so chunks pipeline across iterations.
                cs = slice(c * chunk, (c + 1) * chunk)
                nc.vector.scalar_tensor_tensor(
                    out=rr[:, cs], in0=scol[:, cs], scalar=0.5,
                    in1=ldc[:, cs], op0=OP.mult, op1=OP.subtract)
                if first:
                    nc.vector.tensor_scalar(M2[:, cs], rr[:, cs], -0.8, None,
                                            OP.mult)
                else:
                    nc.vector.scalar_tensor_tensor(
                        out=M2[:, cs], in0=rr[:, cs], scalar=-0.8,
                        in1=M2[:, cs], op0=OP.mult, op1=OP.add)

            # ---- iteration 1: W = clip(2*a, 0, 2*pmax) ----
            for c in range(nch):
                nc.vector.tensor_scalar(
                    w[:, c * CFD:(c + 1) * CFD], a[:, c * CFD:(c + 1) * CFD],
                    2.0, None, OP.mult)
                for gi in range(chunk):
                    g = c * chunk + gi
                    nc.vector.scalar_tensor_tensor(
                        out=w[:, gsl(g)], in0=w[:, gsl(g)], scalar=0.0,
                        in1=hib2[:, :], op0=OP.max, op1=OP.min,
                        accum_out=scol[:, g:g + 1])
                lam_rows(True, c)

            # ---- iterations 2..12 ----
            for _ in range(n_iter - 1):
                for c in range(nch):
                    t = tch.tile([128, CFD], F32, tag="t")
                    # r1n = fl(a - 0.5*W)
                    nc.vector.scalar_tensor_tensor(
                        out=t[:, :].rearrange("p (c f) -> p c f", c=chunk),
                        in0=c3d(w, c), scalar=-0.5, in1=c3d(a, c),
                        op0=OP.mult, op1=OP.add)
                    for gi in range(chunk):
                        g = c * chunk + gi
                        ts = slice(gi * NG, (gi + 1) * NG)
                        # t = fl(r1n + M2) on ScalarE (bit-exact Identity)
                        nc.scalar.activation(
                            t[:, ts], t[:, ts],
                            mybir.ActivationFunctionType.Identity,
                            bias=M2[:, g:g + 1], scale=1.0)
                    # Wr = fl(t + W)   (in-place on W)
                    nc.vector.tensor_tensor(
                        c3d(w, c),
                        t[:, :].rearrange("p (c f) -> p c f", c=chunk),
                        c3d(w, c), OP.add)
                    for gi in range(chunk):
                        g = c * chunk + gi
                        # W = min(max(Wr, 0), 2*pmax), accum row-sum of W
                        nc.vector.scalar_tensor_tensor(
                            out=w[:, gsl(g)], in0=w[:, gsl(g)], scalar=0.0,
                            in1=hib2[:, :], op0=OP.max, op1=OP.min,
                            accum_out=scol[:, g:g + 1])
                    lam_rows(False, c)

            # ---- final slack redistribution ----
            # mask = (W > 2*(pmin+eps)) & (W < 2*(pmax-eps)); ssum = sum(mask)
            thib2_b = thib2[:, :].rearrange("p (o f) -> p o f",
                                            o=1).to_broadcast((128, chunk, NG))
            for c in range(nch):
                t1 = tch.tile([128, CFD], F32, tag="t")
                nc.vector.tensor_tensor(
                    t1[:, :].rearrange("p (c f) -> p c f", c=chunk),
                    c3d(w, c), thib2_b, OP.is_lt)
                for gi in range(chunk):
                    g = c * chunk + gi
                    ts = slice(gi * NG, (gi + 1) * NG)
                    nc.vector.scalar_tensor_tensor(
                        out=a[:, gsl(g)], in0=w[:, gsl(g)], scalar=TLO2,
                        in1=t1[:, ts], op0=OP.is_gt, op1=OP.mult,
                        accum_out=ssum[:, g:g + 1])
            # d = fl(res / max(ssum,1)) with res = fl(s12 - load) (== rr)
            nc.vector.tensor_scalar(ssum[:], ssum[:], 1.0, None, OP.max)
            nc.vector.reciprocal(ssum[:], ssum[:])
            nc.vector.tensor_tensor(dcol[:], rr[:], ssum[:], OP.mult)
            nc.vector.tensor_scalar(dneg2[:], dcol[:], -1.0, None, OP.mult)
            # p_out = fl(0.5*W + (-d)*mask). (-d)*mask is exact (mask is
            # 0/1, on ScalarE via per-partition scale); 0.5*W is exact, so one
            # fused STT reproduces the reference's fl(p - (res/ssum)*mask).
            for g in range(G):
                nc.scalar.activation(
                    a[:, gsl(g)], a[:, gsl(g)],
                    mybir.ActivationFunctionType.Copy,
                    scale=dneg2[:, g:g + 1])
            for c in range(nch):
                nc.vector.scalar_tensor_tensor(
                    out=c3d(w, c), in0=c3d(w, c), scalar=0.5, in1=c3d(a, c),
                    op0=OP.mult, op1=OP.add)
                nc.sync.dma_start(out_d[:, c * CFD:(c + 1) * CFD],
                                  w[:, c * CFD:(c + 1) * CFD])
    nc.compile()
    return nc


_NC_CACHE = {}


def _get_nc(G, num_devices):
    key = (G, num_devices)
    if key not in _NC_CACHE:
        _NC_CACHE[key] = build_nc(G, num_devices)
    return _NC_CACHE[key]


def _prep_inputs(p_pred, total_load, p_max, n_cores, G):
    f = np.float32
    a = np.ascontiguousarray(
        p_pred.astype(f, copy=False).reshape(n_cores, G, 128, NG)
        .transpose(0, 2, 1, 3)).reshape(n_cores, 128, G * NG)
    ld = np.ascontiguousarray(
        total_load.astype(f, copy=False).reshape(n_cores, G, 128)
        .transpose(0, 2, 1))
    pm = p_max.astype(f, copy=False)
    hib2 = np.ascontiguousarray(
        np.broadcast_to((f(2) * pm)[None, :], (128, NG)))
    thib2 = np.ascontiguousarray(
        np.broadcast_to((f(2) * (pm - f(SLACK_EPS)))[None, :], (128, NG)))
    return [dict(a=a[c], loadc=ld[c], hib2=hib2, thib2=thib2)
            for c in range(n_cores)]


def _unpack(results, n_cores, G):
    outs = [r["out"].reshape(128, G, NG).transpose(1, 0, 2).reshape(G * 128, NG)
            for r in results]
    return np.ascontiguousarray(np.concatenate(outs, axis=0))


def run(p_pred, total_load, p_min, p_max, **spmd_kwargs):
    assert p_pred.shape == (B_FULL, NG)
    assert not np.any(p_min), "kernel is specialized for p_min == 0"
    G = B_FULL // (N_CORES * 128)
    nc = _get_nc(G, N_CORES)
    in_maps = _prep_inputs(p_pred, total_load, p_max, N_CORES, G)
    res = run_bass_kernel_spmd(nc, in_maps, list(range(N_CORES)), **spmd_kwargs)
    return _unpack(res.results, N_CORES, G), res


def kernel(p_pred, total_load, p_min, p_max):
    return run(p_pred, total_load, p_min, p_max)[0]
